# revision 14
# baseline (speedup 1.0000x reference)
"""Dense-MoE FFN kernel v3 for TRN2 — expert-parallel, sparse token dispatch.

Each of the 8 cores owns one expert and processes only the tokens routed
to it (host-side gather, capacity 992 = exact max for the fixed input).

v3 changes vs v2 (trace-driven):
- DMA issue cost is ~600ns of *engine* time per dma_start; the v2 ramp
  was issue-rate-paced (131 small DMAs on one ring = 213GB/s). v3 uses
  host-prepped SBUF-image layouts so every stream is a few big DMAs:
  w1 as 32x256KB, w2 as 8x1MB, xt as 2 DMAs/chunk.
- Three parallel DMA queues: w1 + out stores on the Sync HWDGE ring,
  xt/b1 on the Scalar HWDGE ring, w2/wvec/b2 on the GpSimd SWDGE queue.
  First matmul unblocks ~2us into a cold start and the w1/w2 streams
  can't stall each other.
- Phase B streams tokens (stationary = w2 tile, moving = h) producing
  yT [H, cap]; the 96-wide tail token tile no longer wastes a full
  512-cycle matmul per (tm,nn) group. Host transposes the output.
- The per-token prob scale is applied by the vector engine as an
  elementwise multiply against a partition-broadcast wvec row.
- A short burst of dummy matmuls on scratch SBUF at iteration start
  keeps the PE HAM clock-gate warm through the DMA ramp, so the real
  matmul stream starts at 2.4GHz instead of 1.2GHz.
"""

import ml_dtypes
import numpy as np

import concourse.mybir as mybir
import concourse.tile as tile
from concourse import bacc
from concourse.bass_utils import run_bass_kernel_spmd

B, S, H, F, E, K = 2, 2048, 1024, 4096, 8, 2
T = B * S

N_CORES = 8
PSA_BUFS = 3
PSB_BUFS = 3
XT_BUFS = 2
OUT_BUFS = 4
N_DUMMY = 6
CAP_SPARSE = 992
CHUNKS_SPARSE = (512, 480)
CHUNKS_FULL = (512, 512)
CHUNKS_MID = (512, 512, 256)
CHUNKS_DENSE = (512,) * 8

F_TILES = F // 128  # 32
H_TILES = H // 128  # 8

f16 = mybir.dt.bfloat16
f32 = mybir.dt.float32


def _build_nc(chunks=CHUNKS_SPARSE, loop_n: int = 0, with_b2: bool = True,
              staggered: bool = False, cold: bool = False, hoist: bool = True):
    """Build the per-core Bass module for sum(chunks) gathered tokens.

    cold=True (bench-only) moves ALL loads inside the barriered loop body,
    so each iteration replays a full cold execution — the differenced
    per-iteration time then measures single-shot behavior.
    """
    cap = sum(chunks)
    nc = bacc.Bacc(None, target_bir_lowering=False)

    # DRAM tensors in SBUF-image layouts (host pre-arranges):
    #   xT    [128, hk, cap]  = x^T gathered, partition-major over H%128
    #   w1    [128, hk*F]     w1[p, hk*F+f] = W1[hk*128+p, f]
    #   w2    [128, fk*H]     w2[p, fk*H+h] = W2[fk*128+p, h]
    xT_d = nc.dram_tensor("xT", [128, H_TILES, cap], f16, kind="ExternalInput")
    w1_d = nc.dram_tensor("w1", [128, H_TILES * F], f16, kind="ExternalInput")
    w2_d = nc.dram_tensor("w2", [128, F_TILES * H], f16, kind="ExternalInput")
    b1_d = nc.dram_tensor("b1T", [128, F_TILES], f32, kind="ExternalInput")
    b2_d = nc.dram_tensor("b2r", [1, H], f16, kind="ExternalInput")
    wv_d = nc.dram_tensor("wvec", [1, cap], f32, kind="ExternalInput")
    out_d = nc.dram_tensor("out", [H, cap], f32, kind="ExternalOutput")

    with tile.TileContext(nc) as tc:
        with (
            tc.tile_pool(name="const", bufs=1) as constp,
            tc.tile_pool(name="xt", bufs=XT_BUFS) as xtp,
            tc.tile_pool(name="h", bufs=1) as hp,
            tc.tile_pool(name="outsb", bufs=OUT_BUFS) as outp,
            tc.tile_pool(name="psA", bufs=PSA_BUFS, space="PSUM") as psA,
            tc.tile_pool(name="psB", bufs=PSB_BUFS, space="PSUM") as psB,
            tc.tile_pool(name="psD", bufs=1, space="PSUM") as psD,
        ):

            def load_xt_chunk(ci, engs):
                """Allocate + DMA the xt tile for chunk ci (2 DMAs)."""
                tc_sz = chunks[ci]
                off = sum(chunks[:ci])
                xt_sb = xtp.tile([128, H_TILES * tc_sz], f16, name="xt_sb")
                half = H_TILES // 2
                for g in range(2):
                    engs[g % len(engs)].dma_start(
                        xt_sb[:, g * half * tc_sz : (g + 1) * half * tc_sz],
                        xT_d[:, g * half : (g + 1) * half, off : off + tc_sz],
                    )
                return xt_sb

            def emit_consts(hoist_xt: bool):
                """Weight/constant loads, ordered so the first-needed
                ~3MB (xt0 + w1 group 0) has the early HBM window to
                itself.

                sync:   xt0 half A, w1 as 32 256KB DMAs (use order),
                        wvec, then w2 as 8 1MB DMAs — single-FIFO order
                        keeps later streams from stealing bandwidth
                        during the latency-critical ramp
                scalar: xt0 half B + b1 (parallel ring); xt of later
                        chunks loads in the body, time-gated behind the
                        previous chunk's 9th gelu on this engine's FIFO
                gpsimd: b2 (rare) + fallback in-body xt chunks
                """
                b2_sb = None
                ones_sb = None
                if with_b2:
                    b2_sb = constp.tile([1, H], f16, name="b2_sb")
                    nc.gpsimd.dma_start(b2_sb[:], b2_d[:])
                    ones_sb = constp.tile([1, 512], f16, name="ones_sb")
                    nc.vector.memset(ones_sb[:], 1.0)

                # xt chunk 0 as 8 per-hk pieces alternating sync/scalar
                # (both rings idle at t0): the first matmul unblocks
                # ~2.5us in and pieces trickle at sub-HAM-window gaps.
                xt_tiles = None
                if hoist_xt:
                    tc0 = chunks[0]
                    xt0 = xtp.tile([128, H_TILES * tc0], f16, name="xt_sb")
                    for hk in range(H_TILES):
                        eng = nc.sync if hk % 2 == 0 else nc.scalar
                        eng.dma_start(
                            xt0[:, hk * tc0 : (hk + 1) * tc0],
                            xT_d[:, hk, 0:tc0],
                        )
                    xt_tiles = [xt0]
                b1_sb = constp.tile([128, F_TILES], f32, name="b1_sb")
                nc.scalar.dma_start(b1_sb[:], b1_d[:])

                # sync queue: w1 ordered by first use, then wvec + w2.
                w1_sb = constp.tile([128, H_TILES * F], f16, name="w1_sb")
                for g in range(4):
                    for hk in range(H_TILES):
                        nc.sync.dma_start(
                            w1_sb[:, hk * F + g * 1024 : hk * F + (g + 1) * 1024],
                            w1_d[:, hk * F + g * 1024 : hk * F + (g + 1) * 1024],
                        )
                wvb_sb = constp.tile([128, cap], f32, name="wvb_sb")
                nc.sync.dma_start(
                    wvb_sb[:], wv_d[0:1, :].to_broadcast((128, cap))
                )
                w2_sb = constp.tile([128, F_TILES * H], f16, name="w2_sb")
                for q in range(8):
                    nc.sync.dma_start(
                        w2_sb[:, q * 4 * H : (q + 1) * 4 * H],
                        w2_d[:, q * 4 * H : (q + 1) * 4 * H],
                    )

                # PE keep-warm scratch
                dumw_sb = constp.tile([128, 128], f16, name="dumw_sb")
                dumx_sb = constp.tile([128, 512], f16, name="dumx_sb")
                nc.vector.memset(dumw_sb[:], 0.5)
                nc.vector.memset(dumx_sb[:], 0.5)

                return (w1_sb, b1_sb, w2_sb, b2_sb, ones_sb, wvb_sb,
                        dumw_sb, dumx_sb, xt_tiles)

            def emit_body(consts):
                (w1_sb, b1_sb, w2_sb, b2_sb, ones_sb, wvb_sb,
                 dumw_sb, dumx_sb, xt_tiles) = consts

                # Dummy matmuls: keep HAM warm through the DMA ramp.
                if N_DUMMY:
                    psd = psD.tile([128, 512], f32, name="psd")
                    for _ in range(N_DUMMY):
                        nc.tensor.matmul(psd[:], dumw_sb[:], dumx_sb[:],
                                         start=True, stop=True)

                off = 0
                xt_next = xt_tiles[0] if xt_tiles is not None else None
                for ci, tc_sz in enumerate(chunks):
                    if xt_next is not None:
                        xt_sb, xt_next = xt_next, None
                    else:
                        xt_sb = load_xt_chunk(ci, [nc.gpsimd])

                    # phase A: h^T = gelu(w1.T @ xT + b1), bf16, F-major
                    h_sb = hp.tile([128, F_TILES * tc_sz], f16, name="h_sb")
                    for fm in range(F_TILES):
                        ps = psA.tile([128, tc_sz], f32, name="psa")
                        for hk in range(H_TILES):
                            nc.tensor.matmul(
                                ps[:],
                                w1_sb[:, hk * F + fm * 128 : hk * F + (fm + 1) * 128],
                                xt_sb[:, hk * tc_sz : (hk + 1) * tc_sz],
                                start=(hk == 0),
                                stop=(hk == H_TILES - 1),
                            )
                        nc.scalar.activation(
                            h_sb[:, fm * tc_sz : (fm + 1) * tc_sz],
                            ps[:],
                            mybir.ActivationFunctionType.Gelu_apprx_tanh,
                            bias=b1_sb[:, fm : fm + 1],
                        )
                        if fm == 8 and ci + 1 < len(chunks):
                            # prefetch next chunk's xt; the scalar FIFO
                            # gates the issue behind this chunk's gelus
                            xt_next = load_xt_chunk(ci + 1, [nc.scalar])

                    # phase B: yT[hn, tok] = sum_fk w2[fk,hn].T @ h[fk,tok]
                    for hn in range(H_TILES):
                        ps = psB.tile([128, tc_sz], f32, name="psb")
                        for fk in range(F_TILES):
                            nc.tensor.matmul(
                                ps[:],
                                w2_sb[:, fk * H + hn * 128 : fk * H + (hn + 1) * 128],
                                h_sb[:, fk * tc_sz : (fk + 1) * tc_sz],
                                start=(fk == 0),
                                stop=(with_b2 is False and fk == F_TILES - 1),
                            )
                        if with_b2:
                            nc.tensor.matmul(
                                ps[:],
                                b2_sb[:, hn * 128 : (hn + 1) * 128],
                                ones_sb[:, :tc_sz],
                                start=False,
                                stop=True,
                            )
                        o_sb = outp.tile([128, 512], f32, name="o_sb")
                        last = (ci == len(chunks) - 1 and hn == H_TILES - 1)
                        if last:
                            # split the final scale+store so the barrier
                            # waits on a short tail, not the full chain
                            hw = (tc_sz // 2 + 63) & ~63
                            for s0, s1 in ((0, hw), (hw, tc_sz)):
                                nc.vector.tensor_tensor(
                                    o_sb[:, s0:s1], ps[:, s0:s1],
                                    wvb_sb[:, off + s0 : off + s1],
                                    mybir.AluOpType.mult,
                                )
                                nc.sync.dma_start(
                                    out_d[hn * 128 : (hn + 1) * 128,
                                          off + s0 : off + s1],
                                    o_sb[:, s0:s1],
                                )
                        else:
                            nc.vector.tensor_tensor(
                                o_sb[:, :tc_sz], ps[:],
                                wvb_sb[:, off : off + tc_sz],
                                mybir.AluOpType.mult,
                            )
                            nc.sync.dma_start(
                                out_d[hn * 128 : (hn + 1) * 128, off : off + tc_sz],
                                o_sb[:, :tc_sz],
                            )
                    off += tc_sz

            if loop_n:
                kw = {}
                if staggered:
                    kw["staggered_reset"] = True
                if cold:
                    with tc.For_i(0, loop_n, 1, **kw):
                        emit_body(emit_consts(hoist_xt=hoist))
                else:
                    consts = emit_consts(hoist_xt=False)
                    with tc.For_i(0, loop_n, 1, **kw):
                        emit_body(consts)
            else:
                emit_body(emit_consts(hoist_xt=hoist))

    nc.compile()
    return nc


_NC_CACHE = {}


def _get_nc(chunks=CHUNKS_SPARSE, with_b2=True):
    key = (chunks, with_b2)
    if key not in _NC_CACHE:
        _NC_CACHE[key] = _build_nc(chunks, with_b2=with_b2)
    return _NC_CACHE[key]


def _route(probs, experts):
    pf = np.asarray(probs, dtype=np.float32).reshape(K, T)
    ef = np.asarray(experts).reshape(K, T)
    idx_list, w_list = [], []
    for c in range(N_CORES):
        m = ef == c
        sel = m.any(axis=0)
        idx = np.nonzero(sel)[0]
        w = (pf * m).sum(axis=0)[idx]
        idx_list.append(idx)
        w_list.append(w.astype(np.float32))
    return idx_list, w_list


def _prep_in_maps(x, probs, experts, w1, b1, w2, b2, cap=CAP_SPARSE, route=None):
    x = np.asarray(x, dtype=np.float32).reshape(T, H)
    xT = np.ascontiguousarray(x.T).astype(ml_dtypes.bfloat16)
    w1f = np.asarray(w1, dtype=np.float32).astype(ml_dtypes.bfloat16)
    w2f = np.asarray(w2, dtype=np.float32).astype(ml_dtypes.bfloat16)
    b1f = np.asarray(b1, dtype=np.float32)
    b2f = np.asarray(b2, dtype=np.float32).astype(ml_dtypes.bfloat16)
    if route is None:
        route = _route(probs, experts)
    idx_list, w_list = route

    in_maps = []
    for c in range(N_CORES):
        idx, w = idx_list[c], w_list[c]
        n = len(idx)
        xsel = np.zeros((H, cap), dtype=ml_dtypes.bfloat16)
        xsel[:, :n] = xT[:, idx]
        wv = np.zeros(cap, dtype=np.float32)
        wv[:n] = w
        in_maps.append(
            {
                # [H, cap] -> [hk, 128, cap] -> [128, hk, cap]
                "xT": np.ascontiguousarray(
                    xsel.reshape(H_TILES, 128, cap).transpose(1, 0, 2)
                ),
                # [H, F] -> [hk, 128, F] -> [128, hk*F]
                "w1": np.ascontiguousarray(
                    w1f[c].reshape(H_TILES, 128, F).transpose(1, 0, 2)
                ).reshape(128, H_TILES * F),
                # [F, H] -> [fk, 128, H] -> [128, fk*H]
                "w2": np.ascontiguousarray(
                    w2f[c].reshape(F_TILES, 128, H).transpose(1, 0, 2)
                ).reshape(128, F_TILES * H),
                "b1T": np.ascontiguousarray(b1f[c].reshape(F_TILES, 128).T),
                "b2r": b2f[c].reshape(1, H),
                "wvec": wv.reshape(1, cap),
            }
        )
    return in_maps


def _unshard(results, route):
    idx_list, _ = route
    out = np.zeros((T, H), dtype=np.float32)
    for c in range(N_CORES):
        idx = idx_list[c]
        out[idx] += results[c]["out"][:, : len(idx)].T
    return out.reshape(B, S, H)


def kernel(x, probs, experts, w1, b1, w2, b2):
    route = _route(probs, experts)
    max_n = max(len(i) for i in route[0])
    if max_n <= CAP_SPARSE:
        chunks = CHUNKS_SPARSE
    elif max_n <= sum(CHUNKS_FULL):
        chunks = CHUNKS_FULL
    elif max_n <= sum(CHUNKS_MID):
        chunks = CHUNKS_MID
    else:
        chunks = CHUNKS_DENSE
    nc = _get_nc(chunks, with_b2=bool(np.any(np.asarray(b2))))
    in_maps = _prep_in_maps(
        x, probs, experts, w1, b1, w2, b2, cap=sum(chunks), route=route
    )
    res = run_bass_kernel_spmd(nc, in_maps, core_ids=list(range(N_CORES)))
    return _unshard(res.results, route)
